# revision 6
# baseline (speedup 1.0000x reference)
"""Trainium2 Bass kernel for nn_MemoryModule (attention read over a memory bank).

reference:  logits = x @ mem^T ; attn = softmax(logits, axis=1) ; out = attn @ mem
shapes:     x [32768, 128], mem [4096, 128] -> out [32768, 128]

Sharding: data-parallel over batch across 8 cores (4096 rows each), memory
replicated.  No collectives needed (forward only).

Per-core algorithm (B=4096 local rows, M=4096, D=128), ACT-bound (~131us of
exp).  Layout: lt [m%128, chunk, b] per group of NB=512 batch columns.

  - PSUM: pA (4 banks) + pB (3 banks) alternate as lt tiles -> ACT ops up
    to N=2048 (10/group vs 11 at 3+3), amortizing the ~352-cycle ACTIVATE
    overhead; pout (1 bank) accumulates mm2's outT [d, b].  Groups 1-7 use
    [2,3,4,3,4,3,4,3,4,2]: ending on a pB op means the next group's first
    mm1 only waits on the second-to-last ACT (slot rotation) and overlaps
    the last one.
  - Group 0 runs 3-wide ops so each A-tile's 4th bank is transpose
    scratch: memT staging batches (PE transpose + DVE f32r copy) ride
    there (one batch early while the PE is cold, two later); groups 1-5
    stage XT two groups ahead in the pB slot right after its op-8 read.
  - One bf16 pt tile per group; DVE accumulates pt into acc4 for the
    softmax denominator (ops split to stay under the 2048-element DVE
    fast-mode limit; the op-1 seed copy runs as uint32 to halve elements).
  - mm2 runs from a FIFO gated three ACT-ops behind emission, so a
    just-queued matmul never parks the in-order PE queue waiting for exp
    output; a group's own chunks additionally wait until its op 4 so the
    outT handoff (normalize) is complete.
  - Finalize per group, emitted when its last mm2 pops (mid next group):
    DVE reciprocal of the replicated column sums (gpsimd
    partition_all_reduce of the bf16 accumulators, started at group end),
    one tensor_tensor multiply normalizing outT straight out of PSUM to
    bf16, DMA-XBAR transpose to [b, d], gpsimd cast to f32, DMA to HBM.
  - The last group computes column sums with 4 ones-matmuls on the idle
    PE and runs its epilogue in two 256-column slices to cut the drain
    tail.
"""

import numpy as np

import concourse.mybir as mybir
import concourse.tile as tile
from concourse import bacc
from concourse.bass_isa import ReduceOp
from concourse.masks import make_identity

B, M, D = 32768, 4096, 128
NCORES = 8
BLOC = B // NCORES  # 4096 rows per core
P = 128
NB = 512            # batch columns per group
NG = BLOC // NB     # 8 groups
MCHUNKS = M // P    # 32

F32 = mybir.dt.float32
F32R = mybir.dt.float32r
U32 = mybir.dt.uint32
BF16 = mybir.dt.bfloat16
EXP = mybir.ActivationFunctionType.Exp

# per-group ACT-op chunk widths (even index -> pA tile, odd -> pB tile)
PAT_G0 = [3] * 10 + [1, 1]
PAT = [2, 3, 4, 3, 4, 3, 4, 3, 4, 2]
MM2_LAG = 3


def build_nc():
    nc = bacc.Bacc(
        "TRN2", target_bir_lowering=False, debug=False, enable_asserts=False
    )
    x = nc.dram_tensor("x", [BLOC, D], F32, kind="ExternalInput").ap()
    mem = nc.dram_tensor("mem", [M, D], F32, kind="ExternalInput").ap()
    out = nc.dram_tensor("out", [BLOC, D], F32, kind="ExternalOutput").ap()

    with tile.TileContext(nc) as tc:
        with (
            tc.tile_pool(name="const", bufs=1) as constp,
            tc.tile_pool(name="pt", bufs=2) as ptp,
            tc.tile_pool(name="acc", bufs=2) as accp,
            tc.tile_pool(name="fin", bufs=2) as finp,
            tc.tile_pool(name="pA", bufs=1, space="PSUM") as pA,
            tc.tile_pool(name="pB", bufs=1, space="PSUM") as pB,
            tc.tile_pool(name="pout", bufs=1, space="PSUM") as pout,
        ):
            ident = constp.tile([P, P], F32)
            make_identity(nc, ident)
            expbias = constp.tile([P, 1], F32)
            nc.vector.memset(expbias, -45.0)
            ones128 = constp.tile([P, P], BF16)
            nc.vector.memset(ones128, 1.0)

            # Natural-layout staging: partition = row%128, free = (chunk, d).
            stage_m = constp.tile([P, MCHUNKS, D], F32)
            mem_t = mem.rearrange("(c p) d -> p c d", p=P)
            stage_x = constp.tile([P, MCHUNKS, D], F32)
            x_t = x.rearrange("(c p) d -> p c d", p=P)
            for dst, src, s in (
                (stage_m, mem_t, slice(0, 4)),
                (stage_x, x_t, slice(0, 4)),
                (stage_m, mem_t, slice(4, 8)),
                (stage_x, x_t, slice(4, 8)),
                (stage_m, mem_t, slice(8, 16)),
                (stage_x, x_t, slice(8, 16)),
                (stage_m, mem_t, slice(16, 32)),
                (stage_x, x_t, slice(16, 32)),
            ):
                nc.sync.dma_start(out=dst[:, s, :], in_=src[:, s, :])

            memT = constp.tile([P, M], F32R)
            XT = constp.tile([P, BLOC], F32R)
            mem_nat = constp.tile([P, MCHUNKS, D], BF16)
            for q in range(MCHUNKS // 4):
                s = slice(4 * q, 4 * q + 4)
                nc.gpsimd.tensor_copy(out=mem_nat[:, s, :], in_=stage_m[:, s, :])

            def stage4(src, dst, q, scratch):
                """Transpose 4 chunks of src into scratch (psum [P, NB] view),
                then DVE-copy (f32->f32r) to dst[:, q*512:(q+1)*512]."""
                for j in range(4):
                    nc.tensor.transpose(
                        scratch[:, j * P : (j + 1) * P], src[:, 4 * q + j, :], ident
                    )
                nc.vector.tensor_copy(
                    out=dst[:, q * 4 * P : (q + 1) * 4 * P], in_=scratch
                )

            # prelude staging: m q0, q1 and x g0 in the free pA/pB slots.
            pre = pA.tile([P, 4, NB], F32, tag="lt", name="pre")
            preB = pB.tile([P, 3, NB], F32, tag="lt", name="preB")
            stage4(stage_m, memT, 0, pre[:, 0, :])
            stage4(stage_x, XT, 0, preB[:, 0, :])
            stage4(stage_m, memT, 1, pre[:, 1, :])

            # g0 A-op scratch windows (single batches while the PE is cold)
            g0_windows = {
                0: [("m", 2)],
                2: [("m", 3)],
                4: [("m", 4), ("m", 5)],
                6: [("m", 6), ("m", 7)],
                8: [("x", 1), ("x", 2)],
            }

            mm2q = []      # FIFO of (pt_tile, mc, outT, g, opseq)
            fin = {}       # g -> (outT, sums_src)
            st = {"g": 0, "t": 0, "seq": 0}

            def acc_op(acc4, pt, mc0, w, first):
                """Accumulate pt[:, mc0:mc0+w, :] into acc4 (DVE ops kept
                under 2048 elements; seed copy as uint32)."""
                if first:
                    src = pt[:, mc0 : mc0 + w, :].bitcast(U32)
                    nc.vector.tensor_copy(out=acc4[:, :w, :].bitcast(U32), in_=src)
                    if w < 4:
                        nc.vector.memset(acc4[:, w:, :], 0)
                elif w == 4:
                    nc.vector.tensor_add(
                        acc4[:, :3, :], acc4[:, :3, :], pt[:, mc0 : mc0 + 3, :]
                    )
                    nc.vector.tensor_add(
                        acc4[:, 3, :], acc4[:, 3, :], pt[:, mc0 + 3, :]
                    )
                else:
                    nc.vector.tensor_add(
                        acc4[:, :w, :], acc4[:, :w, :], pt[:, mc0 : mc0 + w, :]
                    )

            def emit_finalize(g):
                """g's last mm2 just issued: normalize outT out of PSUM
                (replicated reciprocal), transpose, cast, store."""
                outT, sums_src = fin.pop(g)
                last = g == NG - 1
                rec = finp.tile([P, NB], F32, tag="rec", name=f"rec_{g}")
                nc.vector.reciprocal(rec, sums_src)
                nslice = 2 if last else 1
                w = NB // nslice
                for s in range(nslice):
                    cs = slice(s * w, (s + 1) * w)
                    u16 = finp.tile([P, w], BF16, tag=f"u16{nslice}",
                                    name=f"u16_{g}_{s}")
                    nc.vector.tensor_mul(u16, outT[:, cs], rec[:, cs])
                    unat = finp.tile([P, w // P, P], BF16, tag=f"unat{nslice}",
                                     name=f"unat_{g}_{s}")
                    nc.sync.dma_start_transpose(out=unat, in_=u16)
                    outf = finp.tile([P, w // P, P], F32, tag=f"outf{nslice}",
                                     name=f"outf_{g}_{s}")
                    eng = nc.vector if last else nc.gpsimd
                    eng.tensor_copy(out=outf, in_=unat)
                    nc.sync.dma_start(
                        out=out[g * NB + s * w : g * NB + (s + 1) * w, :].rearrange(
                            "(j p) d -> p j d", p=P
                        ),
                        in_=outf,
                    )

            def issue_mm2(budget):
                issued = 0
                while mm2q and issued < budget:
                    qpt, qmc, qoutT, qg, qseq = mm2q[0]
                    # lag gate: never issue a matmul whose exp output is
                    # still in flight (it would park the in-order PE queue)
                    if qseq > st["seq"] - MM2_LAG:
                        break
                    # handoff gate: a group's own chunks wait until op 4
                    if qg == st["g"] and st["t"] < 3:
                        break
                    mm2q.pop(0)
                    nc.tensor.matmul(
                        qoutT,
                        mem_nat[:, qmc, :],
                        qpt[:, qmc, :],
                        start=(qmc == 0),
                        stop=(qmc == MCHUNKS - 1),
                        skip_group_check=True,
                    )
                    issued += 1
                    if qmc == MCHUNKS - 1:
                        emit_finalize(qg)

            for g in range(NG):
                pat = PAT_G0 if g == 0 else PAT
                st["g"] = g
                xtg = XT[:, g * NB : (g + 1) * NB]
                outT = pout.tile([P, NB], F32, tag="pb", name=f"outT_{g}")
                acc4 = accp.tile([P, 4, NB], BF16, tag="acc", name=f"acc_{g}")
                pt = ptp.tile([P, MCHUNKS, NB], BF16, tag="pt", name=f"pt_{g}")
                mc0 = 0
                for t, w in enumerate(pat):
                    st["t"] = t
                    if t % 2 == 0:
                        lt = pA.tile([P, 4, NB], F32, tag="lt", name=f"ltA_{g}_{t}")
                    else:
                        lt = pB.tile([P, 3, NB], F32, tag="lt", name=f"ltB_{g}_{t}")
                    for c in range(w):
                        mc = mc0 + c
                        nc.tensor.matmul(
                            lt[:, c, :],
                            memT[:, mc * P : (mc + 1) * P],
                            xtg,
                            start=True,
                            stop=True,
                        )
                    # staging rides after this op's mm1 stream
                    if g == 0 and t in g0_windows:
                        for kind, q in g0_windows[t]:
                            if kind == "m":
                                stage4(stage_m, memT, q, lt[:, 3, :])
                            else:
                                stage4(stage_x, XT, q, lt[:, 3, :])
                    if g >= 1 and t == 7 and g + 2 < NG:
                        # after op8's mm1: its pB tile frees at ACT-op8 ->
                        # stage XT two groups out (g0 covered x1, x2)
                        tp = pB.tile([P, 3, NB], F32, tag="lt", name=f"tpx_{g}")
                        stage4(stage_x, XT, g + 2, tp[:, 0, :])
                    nc.scalar.activation(
                        pt[:, mc0 : mc0 + w, :], lt[:, :w, :], EXP, bias=expbias
                    )
                    acc_op(acc4, pt, mc0, w, first=(t == 0))
                    for c in range(w):
                        mm2q.append((pt, mc0 + c, outT, g, st["seq"]))
                    issue_mm2(1 if g == 0 else 4)
                    mc0 += w
                    st["seq"] += 1

                if g < NG - 1:
                    # column sums via gpsimd all-reduce (replicated output)
                    acc2 = finp.tile([P, 2, NB], BF16, tag="acc2", name=f"acc2_{g}")
                    nc.vector.tensor_add(acc2, acc4[:, 0:2, :], acc4[:, 2:4, :])
                    accf = finp.tile([P, NB], BF16, tag="accf", name=f"accf_{g}")
                    nc.vector.tensor_add(accf, acc2[:, 0, :], acc2[:, 1, :])
                    sums = finp.tile([P, NB], BF16, tag="sums", name=f"sums_{g}")
                    nc.gpsimd.partition_all_reduce(sums, accf, P, ReduceOp.add)
                    fin[g] = (outT, sums)
                else:
                    # tail: PE is free -> accumulate column sums in PSUM
                    sums_ps = pB.tile([P, NB], F32, tag="lt", name="sums_ps")
                    for c in range(4):
                        nc.tensor.matmul(
                            sums_ps,
                            ones128,
                            acc4[:, c, :],
                            start=(c == 0),
                            stop=(c == 3),
                            skip_group_check=True,
                        )
                    fin[g] = (outT, sums_ps)

            # drain the software pipeline (gates off)
            st["g"] = NG
            st["seq"] += MM2_LAG
            while mm2q:
                issue_mm2(len(mm2q))

    nc.compile()
    return nc


_NC_CACHE = None


def _get_nc():
    global _NC_CACHE
    if _NC_CACHE is None:
        _NC_CACHE = build_nc()
    return _NC_CACHE


def _in_maps(local_stats, memory):
    local_stats = np.ascontiguousarray(local_stats, dtype=np.float32)
    memory = np.ascontiguousarray(memory, dtype=np.float32)
    return [
        {
            "x": np.ascontiguousarray(local_stats[i * BLOC : (i + 1) * BLOC]),
            "mem": memory,
        }
        for i in range(NCORES)
    ]


def run_spmd(local_stats, memory, **kwargs):
    """Run on all 8 cores; returns BassKernelResults (for test harness use)."""
    from concourse.bass_utils import run_bass_kernel_spmd

    nc = _get_nc()
    return run_bass_kernel_spmd(
        nc, _in_maps(local_stats, memory), core_ids=list(range(NCORES)), **kwargs
    )


def kernel(local_stats, memory):
    res = run_spmd(local_stats, memory)
    return np.concatenate([r["out"] for r in res.results], axis=0)


# revision 11
# speedup vs baseline: 1.2505x; 1.2505x over previous
"""Trainium2 Bass kernel for nn_MemoryModule (attention read over a memory bank).

reference:  logits = x @ mem^T ; attn = softmax(logits, axis=1) ; out = attn @ mem
shapes:     x [32768, 128], mem [4096, 128] -> out [32768, 128]

Sharding: data-parallel over batch across 8 cores (4096 rows each), memory
replicated.  No collectives needed (forward only).

Per-core algorithm (B=4096 local rows, M=4096, D=128), ACT-bound (~131us of
exp).  Layout: lt [m%128, chunk, b] per group of NB=512 batch columns.

  - PSUM: pA (4 banks) + pB (3 banks) alternate as lt tiles -> ACT ops up
    to N=2048 (10/group vs 11 at 3+3), amortizing the ~352-cycle ACTIVATE
    overhead; pout (1 bank) accumulates mm2's outT [d, b].  Groups 1-7 use
    [2,3,4,3,4,3,4,3,4,2]: ending on a pB op means the next group's first
    mm1 only waits on the second-to-last ACT (slot rotation) and overlaps
    the last one.
  - Group 0 runs 3-wide ops so each A-tile's 4th bank is transpose
    scratch: memT staging batches (PE transpose + DVE f32r copy) ride
    there (one batch early while the PE is cold, two later); groups 1-5
    stage XT two groups ahead in the pB slot right after its op-8 read.
  - One bf16 pt tile per group; DVE accumulates pt into acc4 for the
    softmax denominator (ops split to stay under the 2048-element DVE
    fast-mode limit; the op-1 seed copy runs as uint32 to halve elements).
  - mm2 runs from a FIFO gated three ACT-ops behind emission, so a
    just-queued matmul never parks the in-order PE queue waiting for exp
    output; a group's own chunks additionally wait until its op 4 so the
    outT handoff (normalize) is complete.
  - Finalize per group, emitted when its last mm2 pops (mid next group):
    DVE reciprocal of the replicated column sums (gpsimd
    partition_all_reduce of the bf16 accumulators, started at group end),
    one tensor_tensor multiply normalizing outT straight out of PSUM to
    bf16, DMA-XBAR transpose to [b, d], gpsimd cast to f32, DMA to HBM.
  - The last group computes column sums with 4 ones-matmuls on the idle
    PE and runs its epilogue in two 256-column slices to cut the drain
    tail.
"""

import numpy as np

import concourse.mybir as mybir
import concourse.tile as tile
from concourse import bacc
from concourse.bass_isa import ReduceOp
from concourse.masks import make_identity

B, M, D = 32768, 4096, 128
NCORES = 8
BLOC = B // NCORES  # 4096 rows per core
P = 128
NB = 512            # batch columns per group
NG = BLOC // NB     # 8 groups
MCHUNKS = M // P    # 32

F32 = mybir.dt.float32
F32R = mybir.dt.float32r
U32 = mybir.dt.uint32
BF16 = mybir.dt.bfloat16
EXP = mybir.ActivationFunctionType.Exp

# per-group ACT-op chunk widths (even index -> pA tile, odd -> pB tile)
PAT_G0 = [3] * 10 + [1, 1]
PAT = [2, 3, 4, 3, 4, 3, 4, 3, 4, 2]
MM2_LAG = 3


def build_nc():
    nc = bacc.Bacc(
        "TRN2", target_bir_lowering=False, debug=False, enable_asserts=False
    )
    x = nc.dram_tensor("x", [BLOC, D], F32, kind="ExternalInput").ap()
    mem = nc.dram_tensor("mem", [M, D], F32, kind="ExternalInput").ap()
    out = nc.dram_tensor("out", [BLOC, D], F32, kind="ExternalOutput").ap()

    with tile.TileContext(nc) as tc:
        with (
            tc.tile_pool(name="const", bufs=1) as constp,
            tc.tile_pool(name="pt", bufs=2) as ptp,
            tc.tile_pool(name="acc", bufs=2) as accp,
            tc.tile_pool(name="fin", bufs=2) as finp,
            tc.tile_pool(name="pA", bufs=1, space="PSUM") as pA,
            tc.tile_pool(name="pB", bufs=1, space="PSUM") as pB,
            tc.tile_pool(name="pout", bufs=1, space="PSUM") as pout,
        ):
            ident = constp.tile([P, P], F32)
            make_identity(nc, ident)
            expbias = constp.tile([P, 1], F32)
            nc.vector.memset(expbias, -45.0)
            ones_bf = constp.tile([P, 1], BF16)
            nc.vector.memset(ones_bf, 1.0)

            # Natural-layout staging: partition = row%128, free = (chunk, d).
            stage_m = constp.tile([P, MCHUNKS, D], F32)
            mem_t = mem.rearrange("(c p) d -> p c d", p=P)
            stage_x = constp.tile([P, MCHUNKS, D], F32)
            x_t = x.rearrange("(c p) d -> p c d", p=P)
            for dst, src, s in (
                (stage_m, mem_t, slice(0, 4)),
                (stage_x, x_t, slice(0, 4)),
                (stage_m, mem_t, slice(4, 8)),
                (stage_x, x_t, slice(4, 8)),
                (stage_m, mem_t, slice(8, 16)),
                (stage_x, x_t, slice(8, 16)),
                (stage_m, mem_t, slice(16, 32)),
                (stage_x, x_t, slice(16, 32)),
            ):
                nc.sync.dma_start(out=dst[:, s, :], in_=src[:, s, :])

            memT = constp.tile([P, M], F32R)
            XT = constp.tile([P, BLOC], F32R)
            mem_nat = constp.tile([P, MCHUNKS, D], BF16)
            for q in range(MCHUNKS // 4):
                s = slice(4 * q, 4 * q + 4)
                nc.gpsimd.tensor_copy(out=mem_nat[:, s, :], in_=stage_m[:, s, :])

            def stage4(src, dst, q, scratch):
                """Transpose 4 chunks of src into scratch (psum [P, NB] view),
                then DVE-copy (f32->f32r) to dst[:, q*512:(q+1)*512]."""
                for j in range(4):
                    nc.tensor.transpose(
                        scratch[:, j * P : (j + 1) * P], src[:, 4 * q + j, :], ident
                    )
                nc.vector.tensor_copy(
                    out=dst[:, q * 4 * P : (q + 1) * 4 * P], in_=scratch
                )

            # prelude staging: m q0, q1 and x g0 in the free pA/pB slots.
            pre = pA.tile([P, 4, NB], F32, tag="lt", name="pre")
            preB = pB.tile([P, 3, NB], F32, tag="lt", name="preB")
            stage4(stage_m, memT, 0, pre[:, 0, :])
            stage4(stage_x, XT, 0, preB[:, 0, :])
            stage4(stage_m, memT, 1, pre[:, 1, :])

            # g0 A-op scratch windows (single batches while the PE is cold)
            g0_windows = {
                0: [("m", 2)],
                2: [("m", 3)],
                4: [("m", 4), ("m", 5)],
                6: [("m", 6), ("m", 7)],
                8: [("x", 1), ("x", 2)],
            }

            mm2q = []      # FIFO of (pt_tile, mc, outT, g, opseq)
            fin_s = {}     # g -> snat   (set at group end, g<7)
            fin_u = {}     # g -> unat   (set at site A)
            st = {"g": 0, "t": 0, "seq": 0}

            def acc_op(acc4, pt, mc0, w, first):
                """Accumulate pt[:, mc0:mc0+w, :] into acc4 (DVE ops kept
                under 2048 elements; seed copy as uint32)."""
                if first:
                    src = pt[:, mc0 : mc0 + w, :].bitcast(U32)
                    nc.vector.tensor_copy(out=acc4[:, :w, :].bitcast(U32), in_=src)
                    if w < 4:
                        nc.vector.memset(acc4[:, w:, :], 0)
                elif w == 4:
                    nc.vector.tensor_add(
                        acc4[:, :3, :], acc4[:, :3, :], pt[:, mc0 : mc0 + 3, :]
                    )
                    nc.vector.tensor_add(
                        acc4[:, 3, :], acc4[:, 3, :], pt[:, mc0 + 3, :]
                    )
                else:
                    nc.vector.tensor_add(
                        acc4[:, :w, :], acc4[:, :w, :], pt[:, mc0 : mc0 + w, :]
                    )

            def site_a(g, outT):
                """g's last mm2 just issued: free outT via bf16 copy, transpose."""
                u16 = finp.tile([P, NB], BF16, tag="u16", name=f"u16_{g}")
                nc.vector.tensor_copy(out=u16, in_=outT)
                unat = finp.tile([P, 4, P], BF16, tag="unat", name=f"unat_{g}")
                nc.sync.dma_start_transpose(out=unat, in_=u16)
                fin_u[g] = unat

            def site_b(g):
                """Normalize+cast+store group g (snat/unat long since ready)."""
                unat = fin_u.pop(g)
                snat = fin_s.pop(g)
                rs4 = finp.tile([P, 4], F32, tag="rs4", name=f"rs4_{g}")
                nc.vector.reciprocal(rs4, snat[:, :, 0])
                outf = finp.tile([P, 4, P], F32, tag="outf", name=f"outf_{g}")
                for j in range(4):
                    nc.vector.tensor_scalar_mul(
                        outf[:, j, :], unat[:, j, :], rs4[:, j : j + 1]
                    )
                nc.sync.dma_start(
                    out=out[g * NB : (g + 1) * NB, :].rearrange(
                        "(j p) d -> p j d", p=P
                    ),
                    in_=outf,
                )

            def issue_mm2(budget):
                issued = 0
                while mm2q and issued < budget:
                    qpt, qmc, qoutT, qg, qseq = mm2q[0]
                    # lag gate: never issue a matmul whose exp output is
                    # still in flight (it would park the in-order PE queue)
                    if qseq > st["seq"] - MM2_LAG:
                        break
                    # handoff gate: a group's own chunks wait until op 5
                    if qg == st["g"] and st["t"] < 4:
                        break
                    mm2q.pop(0)
                    nc.tensor.matmul(
                        qoutT,
                        mem_nat[:, qmc, :],
                        qpt[:, qmc, :],
                        start=(qmc == 0),
                        stop=(qmc == MCHUNKS - 1),
                        skip_group_check=True,
                    )
                    issued += 1
                    if qmc == MCHUNKS - 1 and qg < NG - 1:
                        site_a(qg, qoutT)
                        if qg >= 1:
                            site_b(qg - 1)

            for g in range(NG):
                pat = PAT_G0 if g == 0 else PAT
                st["g"] = g
                xtg = XT[:, g * NB : (g + 1) * NB]
                outT = pout.tile([P, NB], F32, tag="pb", name=f"outT_{g}")
                acc4 = accp.tile([P, 4, NB], BF16, tag="acc", name=f"acc_{g}")
                pt = ptp.tile([P, MCHUNKS, NB], BF16, tag="pt", name=f"pt_{g}")
                mc0 = 0
                for t, w in enumerate(pat):
                    st["t"] = t
                    if t % 2 == 0:
                        lt = pA.tile([P, 4, NB], F32, tag="lt", name=f"ltA_{g}_{t}")
                    else:
                        lt = pB.tile([P, 3, NB], F32, tag="lt", name=f"ltB_{g}_{t}")
                    for c in range(w):
                        mc = mc0 + c
                        nc.tensor.matmul(
                            lt[:, c, :],
                            memT[:, mc * P : (mc + 1) * P],
                            xtg,
                            start=True,
                            stop=True,
                        )
                    # staging rides after this op's mm1 stream
                    if g == 0 and t in g0_windows:
                        for kind, q in g0_windows[t]:
                            if kind == "m":
                                stage4(stage_m, memT, q, lt[:, 3, :])
                            else:
                                stage4(stage_x, XT, q, lt[:, 3, :])
                    if g >= 1 and t == 7 and g + 2 < NG:
                        # after op8's mm1: its pB tile frees at ACT-op8 ->
                        # stage XT two groups out (g0 covered x1, x2)
                        tp = pB.tile([P, 3, NB], F32, tag="lt", name=f"tpx_{g}")
                        stage4(stage_x, XT, g + 2, tp[:, 0, :])
                    nc.scalar.activation(
                        pt[:, mc0 : mc0 + w, :], lt[:, :w, :], EXP, bias=expbias
                    )
                    acc_op(acc4, pt, mc0, w, first=(t == 0))
                    for c in range(w):
                        mm2q.append((pt, mc0 + c, outT, g, st["seq"]))
                    issue_mm2(1 if g == 0 else 4)
                    mc0 += w
                    st["seq"] += 1

                if g < NG - 1:
                    # column sums via gpsimd all-reduce + XBAR transpose
                    acc2 = finp.tile([P, 2, NB], BF16, tag="acc2", name=f"acc2_{g}")
                    nc.vector.tensor_add(acc2, acc4[:, 0:2, :], acc4[:, 2:4, :])
                    accf = finp.tile([P, NB], BF16, tag="accf", name=f"accf_{g}")
                    nc.vector.tensor_add(accf, acc2[:, 0, :], acc2[:, 1, :])
                    sums = finp.tile([P, NB], BF16, tag="sums", name=f"sums_{g}")
                    nc.gpsimd.partition_all_reduce(sums, accf, P, ReduceOp.add)
                    snat = finp.tile([P, 4, P], BF16, tag="snat", name=f"snat_{g}")
                    nc.sync.dma_start_transpose(out=snat, in_=sums)
                    fin_s[g] = snat
                else:
                    # tail: PE is free -> per-column sums directly in
                    # transposed form: se[:, j] += acc4-block^T @ ones
                    se_ps = pA.tile([P, 4, NB], F32, tag="lt", name="se_ps")
                    for j in range(4):
                        for c in range(4):
                            nc.tensor.matmul(
                                se_ps[:, 0, j : j + 1],
                                acc4[:, c, j * P : (j + 1) * P],
                                ones_bf,
                                start=(c == 0),
                                stop=(c == 3),
                                skip_group_check=True,
                            )
                    tail_outT = outT
                    tail_se = se_ps

            # drain the software pipeline (gates off)
            st["g"] = NG
            st["seq"] += MM2_LAG
            while mm2q:
                issue_mm2(len(mm2q))
            site_b(NG - 2)

            # tail epilogue in two 256-column slices to cut the serial drain
            g = NG - 1
            rs4 = finp.tile([P, 4], F32, tag="rs4", name="rs4_7")
            nc.vector.reciprocal(rs4, tail_se[:, 0, 0:4])
            for s in range(2):
                w = NB // 2
                u16 = finp.tile([P, w], BF16, tag="u16t", name=f"u16t_{s}")
                nc.vector.tensor_copy(out=u16, in_=tail_outT[:, s * w : (s + 1) * w])
                unat = finp.tile([P, 2, P], BF16, tag="unatt", name=f"unatt_{s}")
                nc.sync.dma_start_transpose(out=unat, in_=u16)
                outf = finp.tile([P, 2, P], F32, tag="outft", name=f"outft_{s}")
                for j in range(2):
                    jj = 2 * s + j
                    nc.vector.tensor_scalar_mul(
                        outf[:, j, :], unat[:, j, :], rs4[:, jj : jj + 1]
                    )
                nc.sync.dma_start(
                    out=out[g * NB + s * w : g * NB + (s + 1) * w, :].rearrange(
                        "(j p) d -> p j d", p=P
                    ),
                    in_=outf,
                )

    nc.compile()
    return nc


_NC_CACHE = None


def _get_nc():
    global _NC_CACHE
    if _NC_CACHE is None:
        _NC_CACHE = build_nc()
    return _NC_CACHE


def _in_maps(local_stats, memory):
    local_stats = np.ascontiguousarray(local_stats, dtype=np.float32)
    memory = np.ascontiguousarray(memory, dtype=np.float32)
    return [
        {
            "x": np.ascontiguousarray(local_stats[i * BLOC : (i + 1) * BLOC]),
            "mem": memory,
        }
        for i in range(NCORES)
    ]


def run_spmd(local_stats, memory, **kwargs):
    """Run on all 8 cores; returns BassKernelResults (for test harness use)."""
    from concourse.bass_utils import run_bass_kernel_spmd

    nc = _get_nc()
    return run_bass_kernel_spmd(
        nc, _in_maps(local_stats, memory), core_ids=list(range(NCORES)), **kwargs
    )


def kernel(local_stats, memory):
    res = run_spmd(local_stats, memory)
    return np.concatenate([r["out"] for r in res.results], axis=0)
